# revision 22
# baseline (speedup 1.0000x reference)
"""DSFusion kernel for 8x TRN2 NeuronCores.

Computation (per reference):
    out_x = x @ Wx.T + bx ; out_y = y @ Wy.T + by
    sp1 = softplus(out_x) ; sp2 = softplus(out_y)
    alpha_x = sp1 + 1 ; alpha_y = sp2 + 1
    alpha_a = sp1*sp2/C + sp1 + sp2 + 1        (algebraic collapse of the
                                                Dempster-Shafer combination --
                                                all S/b/u/conflict terms cancel)

Sharding: data-parallel over the batch dim, 1024 rows per core; weights and
biases replicated. Host pre-transposes x/y/W so the contraction dim sits on
SBUF partitions, pre-casts matmul operands to bf16 (fp32 PSUM accumulate),
and packs each DMA stream into one dram tensor per trigger queue so data
arrives in exact consumption order:
  sync   queue: (wx chunk-pair | x tiles 0-3) x8, then x tiles 4-7
  scalar queue: biases, (y tiles 0-3 | wy half-0) x8, wy half-1, y tiles 4-7
  gpsimd queue: all output stores

Schedule: row-tile units [4, 2, 1, 1]. The first X phase covers 4 row tiles
so its weight-consumption rate (~220 GB/s) stays under the ~360 GB/s DMA
roofline while wx streams in; its Y phase runs column-half-major so PSUM
banks freed by the X epilogue are reused without stalling. The final tile's
Y phase is split [250,250,250,125,125] columns so only a ~125-col epilogue
chain is tail-exposed. No PE warm-up: the first matmuls ride the HAM clock
ramp while the DMA prefix streams.
"""

import numpy as np
import ml_dtypes

BATCH = 8192
DIM = 2048
CLASSES = 1000
NCORES = 8
R = BATCH // NCORES          # rows per core (1024)
P = 128
KCH = DIM // P               # contraction chunks (16)
NH = CLASSES // 2            # psum half (500, fits one 2KB bank)
TW = 512                     # rows covered by one x/y stream group (4 tiles)

_CACHE = {}

# Results of the last device run (for the test harness to inspect timing).
LAST_RESULTS = None


def _split_waits(nc, limit=1):
    """The installed walrus can't lower an instruction carrying more than one
    sync wait. Hoist extra waits onto single-wait NOPs inserted immediately
    before the instruction on the same engine (program order preserves the
    wait-all semantics)."""
    import concourse.mybir as mybir

    for f in nc.m.functions:
        for bb in f.blocks:
            out = []
            changed = False
            for ins in list(bb.instructions):
                si = ins.sync_info
                if si is not None and len(si.on_wait) > limit:
                    waits = list(si.on_wait)
                    extra, keep = waits[:-limit], waits[-limit:]
                    for i, w in enumerate(extra):
                        nop = mybir.InstNoOp(name=f"{ins.name}-ws{i}", ins=[], outs=[])
                        nop.engine = ins.engine
                        nop.sync_info = mybir.SyncInfo(on_wait=[w], on_update=[])
                        out.append(nop)
                    ins.sync_info = mybir.SyncInfo(
                        on_wait=keep, on_update=list(si.on_update)
                    )
                    changed = True
                out.append(ins)
            if changed:
                bb.instructions = out


def _build_nc():
    import concourse.bass as bass
    import concourse.mybir as mybir
    import concourse.tile as tile
    from concourse.vector_clock import ScopedClock, VectorClock

    class LeanTailTileContext(tile.TileContext):
        """Tile's stock tail is drain + two all-engine barriers + sem clears;
        with the single-wait-per-instruction legalization the barrier waits
        explode into a ~10us serial EVSEM parade. Replace with: SP drain
        (single-wait NOPs), a two-semaphore handshake barrier (one wait per
        engine), then gpsimd range-clears everything last."""

        def _drain_and_barrier(self, tick_clock, wait_clock):
            nc = self.nc
            vc = tick_clock.global_clock
            n = len(vc)
            for proc in range(n):
                t = vc[proc]
                if t > 0:
                    nop = nc.sync.nop(nofuse=True, hint=f"tail_wait_{proc}")
                    req = ScopedClock(
                        {None: VectorClock([t if i == proc else 0 for i in range(n)])}
                    )
                    wait_clock.add_sem_waits(nop.ins, req)
            nc.sync.drain()

            semB = nc.alloc_semaphore("tail_barrier_b")
            semC = nc.alloc_semaphore("tail_barrier_c")
            engines = list(nc.engines.values())
            pool_eng = nc.gpsimd
            n_eng = len(engines)
            for e in engines:
                e.nop(nofuse=True, hint="tailb_inc").then_inc(semB, 1)
            for e in engines:
                e.wait_ge(semB, n_eng)
            for e in engines:
                if e is not pool_eng:
                    e.nop(nofuse=True, hint="tailc_inc").then_inc(semC, 1)
            pool_eng.wait_ge(semC, n_eng - 1)

            assert self.sems is not None
            popped = self.nc._tile_sem_poison_stack.pop()
            assert popped is self._sem_poison
            nc.clear_and_free_semaphores(
                list(self.sems.allocated().values()) + [semB, semC]
            )

    dt = mybir.dt

    nc = bass.Bass()

    # packed input streams (see module docstring)
    px = nc.dram_tensor("px", [P, KCH // 2, 2, CLASSES + TW], dt.bfloat16,
                        kind="ExternalInput")
    pya = nc.dram_tensor("pya", [P, KCH // 2, 2, NH + TW], dt.bfloat16,
                         kind="ExternalInput")
    pw1 = nc.dram_tensor("pw1", [P, KCH // 4, 4, NH], dt.bfloat16,
                         kind="ExternalInput")
    pxb = nc.dram_tensor("pxb", [P, KCH // 4, 4, TW], dt.bfloat16,
                         kind="ExternalInput")
    pyb = nc.dram_tensor("pyb", [P, KCH // 4, 4, TW], dt.bfloat16,
                         kind="ExternalInput")
    bxy = nc.dram_tensor("bxy", [P, 2, CLASSES], dt.bfloat16,
                         kind="ExternalInput")

    aa_d = nc.dram_tensor("alpha_a", [R, CLASSES], dt.float32, kind="ExternalOutput")
    ax_d = nc.dram_tensor("alpha_x", [R, CLASSES], dt.float32, kind="ExternalOutput")
    ay_d = nc.dram_tensor("alpha_y", [R, CLASSES], dt.float32, kind="ExternalOutput")

    aa3 = aa_d.rearrange("(t p) c -> t p c", p=P)
    ax3 = ax_d.rearrange("(t p) c -> t p c", p=P)
    ay3 = ay_d.rearrange("(t p) c -> t p c", p=P)

    # softplus(x) = ln(exp(x) + 1); the installed ACT tables have no direct
    # softplus, but exp and ln share one table set. Pre-activation values are
    # within +-4 so exp cannot overflow.
    EXP = mybir.ActivationFunctionType.Exp
    LN = mybir.ActivationFunctionType.Ln
    ADD = mybir.AluOpType.add
    MULT = mybir.AluOpType.mult

    with LeanTailTileContext(nc) as tc:
        with (
            tc.tile_pool(name="inpool", bufs=1) as inpool,
            tc.tile_pool(name="epool", bufs=1) as epool,
            tc.tile_pool(name="t2pool", bufs=3) as t2pool,
            tc.tile_pool(name="opool", bufs=2) as opool,
            tc.tile_pool(name="psum", bufs=1, space="PSUM") as ppool,
        ):
            # -- input DMAs (program order = queue order = arrival order) ---
            # sync queue: px (X ramp, ungated), then wy-h1 + x bulk (gated).
            px_sb, px_dma = [], []
            px0b_dma = None
            for kk in range(KCH // 2):
                t_ = inpool.tile([P, 2, CLASSES + TW], dt.bfloat16, tag=f"px{kk}")
                if kk == 0:
                    # chunk-granular first pieces: the very first transfer
                    # rides the DMA pipe alone so matmul 0 starts ~2us sooner
                    px_dma.append(nc.sync.dma_start(t_[:, 0], px[:, 0, 0]))
                    px0b_dma = nc.sync.dma_start(t_[:, 1], px[:, 0, 1])
                else:
                    px_dma.append(nc.sync.dma_start(t_[:], px[:, kk]))
                px_sb.append(t_)
            pw1_sb, pw1_dma = [], []
            for kk in range(KCH // 4):
                t_ = inpool.tile([P, 4, NH], dt.bfloat16, tag=f"pw1{kk}")
                pw1_dma.append(nc.sync.dma_start(t_[:], pw1[:, kk]))
                pw1_sb.append(t_)
            pxb_sb, pxb_dma = [], []
            for kk in range(KCH // 4):
                t_ = inpool.tile([P, 4, TW], dt.bfloat16, tag=f"pxb{kk}")
                pxb_dma.append(nc.sync.dma_start(t_[:], pxb[:, kk]))
                pxb_sb.append(t_)
            pyb_sb, pyb_dma = [], []
            for kk in range(KCH // 4):
                t_ = inpool.tile([P, 4, TW], dt.bfloat16, tag=f"pyb{kk}")
                pyb_dma.append(nc.sync.dma_start(t_[:], pyb[:, kk]))
                pyb_sb.append(t_)

            # scalar queue: bias + (y tiles 0-3 | wy-h0)
            bxy_sb = inpool.tile([P, 2, CLASSES], dt.bfloat16, tag="bxy")
            bxy_dma = nc.scalar.dma_start(bxy_sb[:], bxy[:])
            pya_sb, pya_dma = [], []
            for kk in range(KCH // 2):
                t_ = inpool.tile([P, 2, NH + TW], dt.bfloat16, tag=f"pya{kk}")
                pya_dma.append(nc.scalar.dma_start(t_[:], pya[:, kk]))
                pya_sb.append(t_)

            def x_sl(k, t):  # stationary [128k, 128 rows] for row tile t
                if t < 4:
                    return px_sb[k // 2][:, k % 2, CLASSES + (t % 4) * P:
                                         CLASSES + (t % 4 + 1) * P]
                return pxb_sb[k // 4][:, k % 4, (t - 4) * P:(t - 3) * P]

            def y_sl(k, t):
                if t < 4:
                    return pya_sb[k // 2][:, k % 2, NH + (t % 4) * P:
                                          NH + (t % 4 + 1) * P]
                return pyb_sb[k // 4][:, k % 4, (t - 4) * P:(t - 3) * P]

            def wx_sl(k, h):  # moving [128k, 500 cols]
                return px_sb[k // 2][:, k % 2, NH * h:NH * (h + 1)]

            def wy_sl(k, cs):  # cs a slice within [0, 1000)
                if cs.stop <= NH:
                    return pya_sb[k // 2][:, k % 2, cs]
                assert cs.start >= NH
                return pw1_sb[k // 4][:, k % 4, cs.start - NH:cs.stop - NH]

            HS = [slice(0, NH), slice(NH, CLASSES)]

            mm_anchor = {}

            def bank(i, w=NH, name=""):
                return ppool.tile([P, w], dt.float32, tag=f"pb{i}", name=name)

            t1 = [epool.tile([P, CLASSES], dt.float32, tag=f"t1_{j}", name=f"t1_{j}")
                  for j in range(4)]
            axs = [epool.tile([P, CLASSES], dt.float32, tag=f"ax_{j}", name=f"ax_{j}")
                   for j in range(4)]

            def x_epilogue(tiles, psx):
                """psx[j][h] psum tiles for row tiles `tiles`. Emits the
                h-major bias adds first so h0 banks free earliest."""
                for h in range(2):
                    for j, t in enumerate(tiles):
                        nc.vector.tensor_tensor(
                            t1[t % 4][:, HS[h]], psx[j][h][:], bxy_sb[:, 0, HS[h]], ADD)
                for j, t in enumerate(tiles):
                    for h in range(2):
                        sp1 = t1[t % 4][:, HS[h]]
                        nc.scalar.activation(sp1, sp1, EXP)
                        nc.scalar.activation(sp1, sp1, LN, bias=1.0)
                        ax = axs[t % 4][:, HS[h]]
                        nc.vector.tensor_scalar_add(ax, sp1, 1.0)
                        nc.gpsimd.dma_start(ax3[t][:, HS[h]], ax)
                        nc.vector.tensor_scalar(sp1, sp1, 1.0 / CLASSES, 1.0, MULT, ADD)

            def y_epilogue(t, cs, psy, i, ay_eng=None, aa_eng=None):
                """One column-slice cs of row tile t's Y output from psum psy.
                ay_eng/aa_eng pick the stores' trigger queues (sync/scalar for
                the late tiles so tail stores issue on separate queues)."""
                w = cs.stop - cs.start
                t2 = t2pool.tile([P, w], dt.float32, tag=f"t2_{i % 3}", name=f"t2_{i}")
                nc.vector.tensor_tensor(t2[:], psy[:], bxy_sb[:, 1, cs], ADD)
                sp2 = t2[:]
                nc.scalar.activation(sp2, sp2, EXP)
                nc.scalar.activation(sp2, sp2, LN, bias=1.0)
                ay = opool.tile([P, w], dt.float32, tag=f"ay_{i % 2}", name=f"ay_{i}")
                nc.scalar.add(ay[:], sp2, 1.0)
                (ay_eng or nc.gpsimd).dma_start(ay3[t][:, cs], ay[:])
                nc.vector.tensor_tensor(sp2, sp2, t1[t % 4][:, cs], MULT)
                aa = opool.tile([P, w], dt.float32, tag=f"aa_{i % 2}", name=f"aa_{i}")
                nc.vector.tensor_tensor(aa[:], sp2, axs[t % 4][:, cs], ADD)
                (aa_eng or nc.gpsimd).dma_start(aa3[t][:, cs], aa[:])

            yep = 0  # y-epilogue counter for scratch-tile rotation

            # ---- unit 0: row tiles 0-3 --------------------------------------
            # X phase, chunk-major; bank(t,h) = 2t+h
            psx0 = [[bank(2 * t + h, name=f"x0_{t}{h}") for h in range(2)]
                    for t in range(4)]
            for k in range(KCH):
                st, sp = k == 0, k == KCH - 1
                for t in range(4):
                    for h in range(2):
                        mm = nc.tensor.matmul(psx0[t][h][:], x_sl(k, t), wx_sl(k, h),
                                              start=st, stop=sp)
                        if k < 2 and t == 1 and h == 1:
                            mm_anchor[("x0m", k)] = mm.ins
                mm_anchor[("x0", k)] = mm.ins
            x_epilogue([0, 1, 2, 3], psx0)

            # Y phase, half-major: pass A = h0 on even banks, pass B = h1 odd
            psyA = [bank(2 * t, name=f"y0a_{t}") for t in range(4)]
            for k in range(KCH):
                st, sp = k == 0, k == KCH - 1
                for t in range(4):
                    mm = nc.tensor.matmul(psyA[t][:], y_sl(k, t), wy_sl(k, HS[0]),
                                          start=st, stop=sp)
                mm_anchor[("y0a", k)] = mm.ins
            for t in range(4):
                y_epilogue(t, HS[0], psyA[t], yep); yep += 1
            psyB = [bank(2 * t + 1, name=f"y0b_{t}") for t in range(4)]
            for k in range(KCH):
                st, sp = k == 0, k == KCH - 1
                for t in range(4):
                    mm = nc.tensor.matmul(psyB[t][:], y_sl(k, t), wy_sl(k, HS[1]),
                                          start=st, stop=sp)
                mm_anchor[("y0b", k)] = mm.ins
            for t in range(4):
                y_epilogue(t, HS[1], psyB[t], yep); yep += 1

            # ---- unit 1: row tiles 4,5 --------------------------------------
            psx1 = [[bank(4 * j + 2 * h, name=f"x1_{j}{h}") for h in range(2)]
                    for j in range(2)]
            for k in range(KCH):
                st, sp = k == 0, k == KCH - 1
                for j in range(2):
                    for h in range(2):
                        mm = nc.tensor.matmul(psx1[j][h][:], x_sl(k, 4 + j),
                                              wx_sl(k, h), start=st, stop=sp)
                mm_anchor[("x1", k)] = mm.ins
            x_epilogue([4, 5], psx1)
            psy1 = [[bank(4 * j + 2 * h + 1, name=f"y1_{j}{h}") for h in range(2)]
                    for j in range(2)]
            for k in range(KCH):
                st, sp = k == 0, k == KCH - 1
                for j in range(2):
                    for h in range(2):
                        mm = nc.tensor.matmul(psy1[j][h][:], y_sl(k, 4 + j),
                                              wy_sl(k, HS[h]), start=st, stop=sp)
                mm_anchor[("y1", k)] = mm.ins
            for j in range(2):
                for h in range(2):
                    y_epilogue(4 + j, HS[h], psy1[j][h], yep); yep += 1

            # ---- unit 2: row tile 6 -----------------------------------------
            psx2 = [[bank(2 * h, name=f"x2_{h}") for h in range(2)]]
            for k in range(KCH):
                st, sp = k == 0, k == KCH - 1
                for h in range(2):
                    mm = nc.tensor.matmul(psx2[0][h][:], x_sl(k, 6), wx_sl(k, h),
                                          start=st, stop=sp)
                mm_anchor[("x2", k)] = mm.ins
            x_epilogue([6], psx2)
            psy2 = [[bank(2 * h + 1, name=f"y2_{h}") for h in range(2)]]
            for k in range(KCH):
                st, sp = k == 0, k == KCH - 1
                for h in range(2):
                    mm = nc.tensor.matmul(psy2[0][h][:], y_sl(k, 6), wy_sl(k, HS[h]),
                                          start=st, stop=sp)
                mm_anchor[("y2", k)] = mm.ins
            for h in range(2):
                y_epilogue(6, HS[h], psy2[0][h], yep, ay_eng=nc.sync); yep += 1

            # ---- unit 3: row tile 7, Y split fine for a short tail ----------
            psx3 = [[bank(4 + 2 * h, name=f"x3_{h}") for h in range(2)]]
            for k in range(KCH):
                st, sp = k == 0, k == KCH - 1
                for h in range(2):
                    mm = nc.tensor.matmul(psx3[0][h][:], x_sl(k, 7), wx_sl(k, h),
                                          start=st, stop=sp)
                mm_anchor[("x3", k)] = mm.ins
            x_epilogue([7], psx3)

            PIECES = [(0, 250), (250, 500), (500, 750), (750, 875), (875, 1000)]
            pbanks = [5, 7, 1, 3, 5]
            for i, (c0, c1) in enumerate(PIECES):
                cs = slice(c0, c1)
                psq = bank(pbanks[i], w=c1 - c0, name=f"y3_{i}")
                for k in range(KCH):
                    st, sp = k == 0, k == KCH - 1
                    mm = nc.tensor.matmul(psq[:], y_sl(k, 7), wy_sl(k, cs),
                                          start=st, stop=sp)
                    if i == 0:
                        mm_anchor[("y3", k)] = mm.ins
                y_epilogue(7, cs, psq, yep, ay_eng=nc.sync,
                           aa_eng=nc.scalar if i == 3 else None); yep += 1

            # -- DMA backpressure: keep the scalar/bulk streams one phase ----
            # behind the PE so the sync ramp owns the DMA engines early.
            from concourse.tile_rust import add_dep_helper

            def _gate(dma, phase, k, why):
                add_dep_helper(dma.ins, mm_anchor[(phase, min(k, KCH - 1))],
                               reason=why)

            # Keep ~2-3 transfers in flight and anchor every gate on the
            # self-contained X0 / early-Y0a chain: same-phase anchors create
            # stall->late-gate->late-data feedback, and a big ungated flood
            # round-robins the DMA engines so every piece lands late.
            _gate(bxy_dma, "x0", 2, "bias stage")
            _gate(px0b_dma, "x0m", 0, "x ramp stage")
            _gate(px_dma[1], "x0m", 0, "x ramp stage")
            _gate(px_dma[2], "x0m", 1, "x ramp stage")
            _gate(px_dma[3], "x0", 2, "x ramp stage")
            for kk in range(4, KCH // 2):
                _gate(px_dma[kk], "x0", 2 * kk - 5, "x ramp stage")
            for kk in range(KCH // 2):
                _gate(pya_dma[kk], "x0", min(13, 7 + kk), "y ramp stage")
            for kk in range(KCH // 4):
                _gate(pw1_dma[kk], "x0", 15, "wy h1 stage")
                _gate(pxb_dma[kk], "y0a", 4, "x bulk stage")
                _gate(pyb_dma[kk], "y0a", 12, "y bulk stage")

    _split_waits(nc)
    return nc


def _trim_walrus_sem_clears():
    """The walrus postamble zeroes all 256 semaphores one instruction at a
    time (~7.5us). Capping the sem space trims the parade; this kernel's
    sems stay below 176."""
    import concourse.bass_utils as bu

    if getattr(bu, "_dsf_sem_patch", False):
        return
    orig = bu.get_walrus_args

    def patched(arch, tmpdir, *, dve_root=None):
        return orig(arch, tmpdir, dve_root=dve_root) + ["--max-sem-num=176"]

    bu.get_walrus_args = patched
    bu._dsf_sem_patch = True


def kernel(x, y, Wx, bx, Wy, by):
    global LAST_RESULTS
    from concourse.bass_utils import run_bass_kernel_spmd

    _trim_walrus_sem_clears()

    if "nc" not in _CACHE:
        _CACHE["nc"] = _build_nc()
    nc = _CACHE["nc"]

    bf16 = ml_dtypes.bfloat16
    x = np.asarray(x, dtype=np.float32)
    y = np.asarray(y, dtype=np.float32)
    xb = x.astype(bf16)                       # [BATCH, DIM]
    yb = y.astype(bf16)
    wxT = np.asarray(Wx, dtype=np.float32).astype(bf16).T  # [DIM, CLASSES]
    wyT = np.asarray(Wy, dtype=np.float32).astype(bf16).T
    KH = KCH // 2

    # [DIM, C] -> [P, KCH, C]  (chunk k occupies rows k*P:(k+1)*P)
    wx3 = np.ascontiguousarray(wxT.reshape(KCH, P, CLASSES).transpose(1, 0, 2))
    wy3 = np.ascontiguousarray(wyT.reshape(KCH, P, CLASSES).transpose(1, 0, 2))

    bxy = np.empty((P, 2, CLASSES), dtype=bf16)
    bxy[:, 0, :] = np.broadcast_to(np.asarray(bx, np.float32).astype(bf16), (P, CLASSES))
    bxy[:, 1, :] = np.broadcast_to(np.asarray(by, np.float32).astype(bf16), (P, CLASSES))

    xT = np.ascontiguousarray(xb.T)           # [DIM, BATCH]
    yT = np.ascontiguousarray(yb.T)
    x4 = xT.reshape(KCH, P, BATCH).transpose(1, 0, 2)   # [P, KCH, BATCH]
    y4 = yT.reshape(KCH, P, BATCH).transpose(1, 0, 2)

    in_maps = []
    for c in range(NCORES):
        rs = slice(c * R, (c + 1) * R)
        xc = x4[:, :, rs]                      # [P, KCH, R]
        yc = y4[:, :, rs]

        px = np.empty((P, KH, 2, CLASSES + TW), dtype=bf16)
        px[:, :, :, :CLASSES] = wx3.reshape(P, KH, 2, CLASSES)
        px[:, :, :, CLASSES:] = xc[:, :, :TW].reshape(P, KH, 2, TW)

        pya = np.empty((P, KH, 2, NH + TW), dtype=bf16)
        pya[:, :, :, :NH] = wy3[:, :, :NH].reshape(P, KH, 2, NH)
        pya[:, :, :, NH:] = yc[:, :, :TW].reshape(P, KH, 2, TW)

        pw1 = np.ascontiguousarray(wy3[:, :, NH:].reshape(P, KCH // 4, 4, NH))
        pxb = np.ascontiguousarray(xc[:, :, TW:].reshape(P, KCH // 4, 4, TW))
        pyb = np.ascontiguousarray(yc[:, :, TW:].reshape(P, KCH // 4, 4, TW))

        in_maps.append(
            {"px": px, "pya": pya, "pw1": pw1, "pxb": pxb, "pyb": pyb, "bxy": bxy}
        )

    res = run_bass_kernel_spmd(nc, in_maps, core_ids=list(range(NCORES)))
    LAST_RESULTS = res

    aa = np.concatenate([res.results[c]["alpha_a"] for c in range(NCORES)], axis=0)
    ax = np.concatenate([res.results[c]["alpha_x"] for c in range(NCORES)], axis=0)
    ay = np.concatenate([res.results[c]["alpha_y"] for c in range(NCORES)], axis=0)
    return (aa, ax, ay)


# revision 24
# speedup vs baseline: 1.0517x; 1.0517x over previous
"""DSFusion kernel for 8x TRN2 NeuronCores.

Computation (per reference):
    out_x = x @ Wx.T + bx ; out_y = y @ Wy.T + by
    sp1 = softplus(out_x) ; sp2 = softplus(out_y)
    alpha_x = sp1 + 1 ; alpha_y = sp2 + 1
    alpha_a = sp1*sp2/C + sp1 + sp2 + 1        (algebraic collapse of the
                                                Dempster-Shafer combination --
                                                all S/b/u/conflict terms cancel)

Sharding: data-parallel over the batch dim, 1024 rows per core; weights and
biases replicated. Host pre-transposes x/y/W so the contraction dim sits on
SBUF partitions, pre-casts matmul operands to bf16 (fp32 PSUM accumulate),
and packs each DMA stream into one dram tensor per trigger queue so data
arrives in exact consumption order:
  sync   queue: (wx chunk-pair | x tiles 0-3) x8, then x tiles 4-7
  scalar queue: biases, (y tiles 0-3 | wy half-0) x8, wy half-1, y tiles 4-7
  gpsimd queue: all output stores

Schedule: row-tile units [4, 2, 1, 1]. The first X phase covers 4 row tiles
so its weight-consumption rate (~220 GB/s) stays under the ~360 GB/s DMA
roofline while wx streams in; its Y phase runs column-half-major so PSUM
banks freed by the X epilogue are reused without stalling. The final tile's
Y phase is split [250,250,250,125,125] columns so only a ~125-col epilogue
chain is tail-exposed. No PE warm-up: the first matmuls ride the HAM clock
ramp while the DMA prefix streams.
"""

import numpy as np
import ml_dtypes

BATCH = 8192
DIM = 2048
CLASSES = 1000
NCORES = 8
R = BATCH // NCORES          # rows per core (1024)
P = 128
KCH = DIM // P               # contraction chunks (16)
NH = CLASSES // 2            # psum half (500, fits one 2KB bank)
TW = 512                     # rows covered by one x/y stream group (4 tiles)

_CACHE = {}

# Results of the last device run (for the test harness to inspect timing).
LAST_RESULTS = None


def _split_waits(nc, limit=1):
    """The installed walrus can't lower an instruction carrying more than one
    sync wait. Hoist extra waits onto single-wait NOPs inserted immediately
    before the instruction on the same engine (program order preserves the
    wait-all semantics)."""
    import concourse.mybir as mybir

    for f in nc.m.functions:
        for bb in f.blocks:
            out = []
            changed = False
            for ins in list(bb.instructions):
                si = ins.sync_info
                if si is not None and len(si.on_wait) > limit:
                    waits = list(si.on_wait)
                    extra, keep = waits[:-limit], waits[-limit:]
                    for i, w in enumerate(extra):
                        nop = mybir.InstNoOp(name=f"{ins.name}-ws{i}", ins=[], outs=[])
                        nop.engine = ins.engine
                        nop.sync_info = mybir.SyncInfo(on_wait=[w], on_update=[])
                        out.append(nop)
                    ins.sync_info = mybir.SyncInfo(
                        on_wait=keep, on_update=list(si.on_update)
                    )
                    changed = True
                out.append(ins)
            if changed:
                bb.instructions = out


def _build_nc():
    import concourse.bass as bass
    import concourse.mybir as mybir
    import concourse.tile as tile
    from concourse.vector_clock import ScopedClock, VectorClock

    class LeanTailTileContext(tile.TileContext):
        """Tile's stock tail is drain + two all-engine barriers + sem clears;
        with the single-wait-per-instruction legalization the barrier waits
        explode into a ~10us serial EVSEM parade. Replace with: SP drain
        (single-wait NOPs), a two-semaphore handshake barrier (one wait per
        engine), then gpsimd range-clears everything last."""

        def _drain_and_barrier(self, tick_clock, wait_clock):
            nc = self.nc
            vc = tick_clock.global_clock
            n = len(vc)
            for proc in range(n):
                t = vc[proc]
                if t > 0:
                    nop = nc.sync.nop(nofuse=True, hint=f"tail_wait_{proc}")
                    req = ScopedClock(
                        {None: VectorClock([t if i == proc else 0 for i in range(n)])}
                    )
                    wait_clock.add_sem_waits(nop.ins, req)
            nc.sync.drain()

            semB = nc.alloc_semaphore("tail_barrier_b")
            semC = nc.alloc_semaphore("tail_barrier_c")
            engines = list(nc.engines.values())
            pool_eng = nc.gpsimd
            n_eng = len(engines)
            for e in engines:
                e.nop(nofuse=True, hint="tailb_inc").then_inc(semB, 1)
            for e in engines:
                e.wait_ge(semB, n_eng)
            for e in engines:
                if e is not pool_eng:
                    e.nop(nofuse=True, hint="tailc_inc").then_inc(semC, 1)
            pool_eng.wait_ge(semC, n_eng - 1)

            assert self.sems is not None
            popped = self.nc._tile_sem_poison_stack.pop()
            assert popped is self._sem_poison
            nc.clear_and_free_semaphores(
                list(self.sems.allocated().values()) + [semB, semC]
            )

    dt = mybir.dt

    nc = bass.Bass()

    # packed input streams (see module docstring)
    px = nc.dram_tensor("px", [P, KCH // 2, 2, CLASSES + TW], dt.bfloat16,
                        kind="ExternalInput")
    pya = nc.dram_tensor("pya", [P, KCH // 2, 2, NH + TW], dt.bfloat16,
                         kind="ExternalInput")
    pw1 = nc.dram_tensor("pw1", [P, KCH // 4, 4, NH], dt.bfloat16,
                         kind="ExternalInput")
    pxb = nc.dram_tensor("pxb", [P, KCH // 4, 4, TW], dt.bfloat16,
                         kind="ExternalInput")
    pyb = nc.dram_tensor("pyb", [P, KCH // 4, 4, TW], dt.bfloat16,
                         kind="ExternalInput")
    bxy = nc.dram_tensor("bxy", [P, 2, CLASSES], dt.bfloat16,
                         kind="ExternalInput")

    aa_d = nc.dram_tensor("alpha_a", [R, CLASSES], dt.float32, kind="ExternalOutput")
    ax_d = nc.dram_tensor("alpha_x", [R, CLASSES], dt.float32, kind="ExternalOutput")
    ay_d = nc.dram_tensor("alpha_y", [R, CLASSES], dt.float32, kind="ExternalOutput")

    aa3 = aa_d.rearrange("(t p) c -> t p c", p=P)
    ax3 = ax_d.rearrange("(t p) c -> t p c", p=P)
    ay3 = ay_d.rearrange("(t p) c -> t p c", p=P)

    # softplus(x) = ln(exp(x) + 1); the installed ACT tables have no direct
    # softplus, but exp and ln share one table set. Pre-activation values are
    # within +-4 so exp cannot overflow.
    EXP = mybir.ActivationFunctionType.Exp
    LN = mybir.ActivationFunctionType.Ln
    ADD = mybir.AluOpType.add
    MULT = mybir.AluOpType.mult

    with LeanTailTileContext(nc) as tc:
        with (
            tc.tile_pool(name="inpool", bufs=1) as inpool,
            tc.tile_pool(name="epool", bufs=1) as epool,
            tc.tile_pool(name="t2pool", bufs=3) as t2pool,
            tc.tile_pool(name="opool", bufs=2) as opool,
            tc.tile_pool(name="psum", bufs=1, space="PSUM") as ppool,
        ):
            # -- input DMAs (program order = queue order = arrival order) ---
            # sync queue: px (X ramp, ungated), then wy-h1 + x bulk (gated).
            px_sb, px_dma = [], []
            for kk in range(KCH // 2):
                t_ = inpool.tile([P, 2, CLASSES + TW], dt.bfloat16, tag=f"px{kk}")
                px_dma.append(nc.sync.dma_start(t_[:], px[:, kk]))
                px_sb.append(t_)
            pw1_sb, pw1_dma = [], []
            for kk in range(KCH // 4):
                t_ = inpool.tile([P, 4, NH], dt.bfloat16, tag=f"pw1{kk}")
                pw1_dma.append(nc.sync.dma_start(t_[:], pw1[:, kk]))
                pw1_sb.append(t_)
            pxb_sb, pxb_dma = [], []
            for kk in range(KCH // 4):
                t_ = inpool.tile([P, 4, TW], dt.bfloat16, tag=f"pxb{kk}")
                pxb_dma.append(nc.sync.dma_start(t_[:], pxb[:, kk]))
                pxb_sb.append(t_)
            pyb_sb, pyb_dma = [], []
            for kk in range(KCH // 4):
                t_ = inpool.tile([P, 4, TW], dt.bfloat16, tag=f"pyb{kk}")
                pyb_dma.append(nc.sync.dma_start(t_[:], pyb[:, kk]))
                pyb_sb.append(t_)

            # scalar queue: bias + (y tiles 0-3 | wy-h0)
            bxy_sb = inpool.tile([P, 2, CLASSES], dt.bfloat16, tag="bxy")
            bxy_dma = nc.scalar.dma_start(bxy_sb[:], bxy[:])
            pya_sb, pya_dma = [], []
            for kk in range(KCH // 2):
                t_ = inpool.tile([P, 2, NH + TW], dt.bfloat16, tag=f"pya{kk}")
                pya_dma.append(nc.scalar.dma_start(t_[:], pya[:, kk]))
                pya_sb.append(t_)

            def x_sl(k, t):  # stationary [128k, 128 rows] for row tile t
                if t < 4:
                    return px_sb[k // 2][:, k % 2, CLASSES + (t % 4) * P:
                                         CLASSES + (t % 4 + 1) * P]
                return pxb_sb[k // 4][:, k % 4, (t - 4) * P:(t - 3) * P]

            def y_sl(k, t):
                if t < 4:
                    return pya_sb[k // 2][:, k % 2, NH + (t % 4) * P:
                                          NH + (t % 4 + 1) * P]
                return pyb_sb[k // 4][:, k % 4, (t - 4) * P:(t - 3) * P]

            def wx_sl(k, h):  # moving [128k, 500 cols]
                return px_sb[k // 2][:, k % 2, NH * h:NH * (h + 1)]

            def wy_sl(k, cs):  # cs a slice within [0, 1000)
                if cs.stop <= NH:
                    return pya_sb[k // 2][:, k % 2, cs]
                assert cs.start >= NH
                return pw1_sb[k // 4][:, k % 4, cs.start - NH:cs.stop - NH]

            HS = [slice(0, NH), slice(NH, CLASSES)]

            mm_anchor = {}

            def bank(i, w=NH, name=""):
                return ppool.tile([P, w], dt.float32, tag=f"pb{i}", name=name)

            t1 = [epool.tile([P, CLASSES], dt.float32, tag=f"t1_{j}", name=f"t1_{j}")
                  for j in range(4)]
            axs = [epool.tile([P, CLASSES], dt.float32, tag=f"ax_{j}", name=f"ax_{j}")
                   for j in range(4)]

            def x_epilogue(tiles, psx):
                """psx[j][h] psum tiles for row tiles `tiles`. Emits the
                h-major bias adds first so h0 banks free earliest."""
                for h in range(2):
                    for j, t in enumerate(tiles):
                        nc.vector.tensor_tensor(
                            t1[t % 4][:, HS[h]], psx[j][h][:], bxy_sb[:, 0, HS[h]], ADD)
                for j, t in enumerate(tiles):
                    for h in range(2):
                        sp1 = t1[t % 4][:, HS[h]]
                        nc.scalar.activation(sp1, sp1, EXP)
                        nc.scalar.activation(sp1, sp1, LN, bias=1.0)
                        ax = axs[t % 4][:, HS[h]]
                        nc.vector.tensor_scalar_add(ax, sp1, 1.0)
                        nc.gpsimd.dma_start(ax3[t][:, HS[h]], ax)
                        nc.vector.tensor_scalar(sp1, sp1, 1.0 / CLASSES, 1.0, MULT, ADD)

            def y_epilogue(t, cs, psy, i, ay_eng=None, aa_eng=None):
                """One column-slice cs of row tile t's Y output from psum psy.
                ay_eng/aa_eng pick the stores' trigger queues (sync/scalar for
                the late tiles so tail stores issue on separate queues)."""
                w = cs.stop - cs.start
                t2 = t2pool.tile([P, w], dt.float32, tag=f"t2_{i % 3}", name=f"t2_{i}")
                nc.vector.tensor_tensor(t2[:], psy[:], bxy_sb[:, 1, cs], ADD)
                sp2 = t2[:]
                nc.scalar.activation(sp2, sp2, EXP)
                nc.scalar.activation(sp2, sp2, LN, bias=1.0)
                ay = opool.tile([P, w], dt.float32, tag=f"ay_{i % 2}", name=f"ay_{i}")
                nc.scalar.add(ay[:], sp2, 1.0)
                (ay_eng or nc.gpsimd).dma_start(ay3[t][:, cs], ay[:])
                nc.vector.tensor_tensor(sp2, sp2, t1[t % 4][:, cs], MULT)
                aa = opool.tile([P, w], dt.float32, tag=f"aa_{i % 2}", name=f"aa_{i}")
                nc.vector.tensor_tensor(aa[:], sp2, axs[t % 4][:, cs], ADD)
                (aa_eng or nc.gpsimd).dma_start(aa3[t][:, cs], aa[:])

            yep = 0  # y-epilogue counter for scratch-tile rotation

            # ---- unit 0: row tiles 0-3 --------------------------------------
            # X phase, chunk-major; bank(t,h) = 2t+h
            psx0 = [[bank(2 * t + h, name=f"x0_{t}{h}") for h in range(2)]
                    for t in range(4)]
            for k in range(KCH):
                st, sp = k == 0, k == KCH - 1
                for t in range(4):
                    for h in range(2):
                        mm = nc.tensor.matmul(psx0[t][h][:], x_sl(k, t), wx_sl(k, h),
                                              start=st, stop=sp)
                mm_anchor[("x0", k)] = mm.ins
            x_epilogue([0, 1, 2, 3], psx0)

            # Y phase, half-major: pass A = h0 on even banks, pass B = h1 odd
            psyA = [bank(2 * t, name=f"y0a_{t}") for t in range(4)]
            for k in range(KCH):
                st, sp = k == 0, k == KCH - 1
                for t in range(4):
                    mm = nc.tensor.matmul(psyA[t][:], y_sl(k, t), wy_sl(k, HS[0]),
                                          start=st, stop=sp)
                mm_anchor[("y0a", k)] = mm.ins
            for t in range(4):
                y_epilogue(t, HS[0], psyA[t], yep); yep += 1
            psyB = [bank(2 * t + 1, name=f"y0b_{t}") for t in range(4)]
            for k in range(KCH):
                st, sp = k == 0, k == KCH - 1
                for t in range(4):
                    mm = nc.tensor.matmul(psyB[t][:], y_sl(k, t), wy_sl(k, HS[1]),
                                          start=st, stop=sp)
                mm_anchor[("y0b", k)] = mm.ins
            for t in range(4):
                y_epilogue(t, HS[1], psyB[t], yep); yep += 1

            # ---- unit 1: row tiles 4,5 --------------------------------------
            psx1 = [[bank(4 * j + 2 * h, name=f"x1_{j}{h}") for h in range(2)]
                    for j in range(2)]
            for k in range(KCH):
                st, sp = k == 0, k == KCH - 1
                for j in range(2):
                    for h in range(2):
                        mm = nc.tensor.matmul(psx1[j][h][:], x_sl(k, 4 + j),
                                              wx_sl(k, h), start=st, stop=sp)
                mm_anchor[("x1", k)] = mm.ins
            x_epilogue([4, 5], psx1)
            psy1 = [[bank(4 * j + 2 * h + 1, name=f"y1_{j}{h}") for h in range(2)]
                    for j in range(2)]
            for k in range(KCH):
                st, sp = k == 0, k == KCH - 1
                for j in range(2):
                    for h in range(2):
                        mm = nc.tensor.matmul(psy1[j][h][:], y_sl(k, 4 + j),
                                              wy_sl(k, HS[h]), start=st, stop=sp)
                mm_anchor[("y1", k)] = mm.ins
            for j in range(2):
                for h in range(2):
                    y_epilogue(4 + j, HS[h], psy1[j][h], yep); yep += 1

            # ---- unit 2: row tile 6 -----------------------------------------
            psx2 = [[bank(2 * h, name=f"x2_{h}") for h in range(2)]]
            for k in range(KCH):
                st, sp = k == 0, k == KCH - 1
                for h in range(2):
                    mm = nc.tensor.matmul(psx2[0][h][:], x_sl(k, 6), wx_sl(k, h),
                                          start=st, stop=sp)
                mm_anchor[("x2", k)] = mm.ins
            x_epilogue([6], psx2)
            psy2 = [[bank(2 * h + 1, name=f"y2_{h}") for h in range(2)]]
            for k in range(KCH):
                st, sp = k == 0, k == KCH - 1
                for h in range(2):
                    mm = nc.tensor.matmul(psy2[0][h][:], y_sl(k, 6), wy_sl(k, HS[h]),
                                          start=st, stop=sp)
                mm_anchor[("y2", k)] = mm.ins
            for h in range(2):
                y_epilogue(6, HS[h], psy2[0][h], yep, ay_eng=nc.sync); yep += 1

            # ---- unit 3: row tile 7, Y split fine for a short tail ----------
            psx3 = [[bank(4 + 2 * h, name=f"x3_{h}") for h in range(2)]]
            for k in range(KCH):
                st, sp = k == 0, k == KCH - 1
                for h in range(2):
                    mm = nc.tensor.matmul(psx3[0][h][:], x_sl(k, 7), wx_sl(k, h),
                                          start=st, stop=sp)
                mm_anchor[("x3", k)] = mm.ins
            x_epilogue([7], psx3)

            PIECES = [(0, 250), (250, 500), (500, 750), (750, 875), (875, 1000)]
            pbanks = [5, 7, 1, 3, 5]
            for i, (c0, c1) in enumerate(PIECES):
                cs = slice(c0, c1)
                psq = bank(pbanks[i], w=c1 - c0, name=f"y3_{i}")
                for k in range(KCH):
                    st, sp = k == 0, k == KCH - 1
                    mm = nc.tensor.matmul(psq[:], y_sl(k, 7), wy_sl(k, cs),
                                          start=st, stop=sp)
                    if i == 0:
                        mm_anchor[("y3", k)] = mm.ins
                y_epilogue(7, cs, psq, yep, ay_eng=nc.sync,
                           aa_eng=nc.scalar if i == 3 else None); yep += 1

            # -- DMA backpressure: keep the scalar/bulk streams one phase ----
            # behind the PE so the sync ramp owns the DMA engines early.
            from concourse.tile_rust import add_dep_helper

            def _gate(dma, phase, k, why):
                add_dep_helper(dma.ins, mm_anchor[(phase, min(k, KCH - 1))],
                               reason=why)

            # Keep ~2-3 transfers in flight and anchor every gate on the
            # self-contained X0 / early-Y0a chain: same-phase anchors create
            # stall->late-gate->late-data feedback, and a big ungated flood
            # round-robins the DMA engines so every piece lands late.
            _gate(bxy_dma, "x0", 2, "bias stage")
            for kk in range(2, KCH // 2):
                _gate(px_dma[kk], "x0", max(0, 2 * kk - 7), "x ramp stage")
            for kk in range(KCH // 2):
                _gate(pya_dma[kk], "x0", min(12, 5 + kk), "y ramp stage")
            for kk in range(KCH // 4):
                _gate(pw1_dma[kk], "x0", 15, "wy h1 stage")
                _gate(pxb_dma[kk], "y0a", 4, "x bulk stage")
                _gate(pyb_dma[kk], "y0a", 12, "y bulk stage")

    _split_waits(nc)
    return nc


def _trim_walrus_sem_clears():
    """The walrus postamble zeroes all 256 semaphores one instruction at a
    time (~7.5us). Capping the sem space trims the parade; this kernel's
    sems stay below 176."""
    import concourse.bass_utils as bu

    if getattr(bu, "_dsf_sem_patch", False):
        return
    orig = bu.get_walrus_args

    def patched(arch, tmpdir, *, dve_root=None):
        return orig(arch, tmpdir, dve_root=dve_root) + ["--max-sem-num=176"]

    bu.get_walrus_args = patched
    bu._dsf_sem_patch = True


def kernel(x, y, Wx, bx, Wy, by):
    global LAST_RESULTS
    from concourse.bass_utils import run_bass_kernel_spmd

    _trim_walrus_sem_clears()

    if "nc" not in _CACHE:
        _CACHE["nc"] = _build_nc()
    nc = _CACHE["nc"]

    bf16 = ml_dtypes.bfloat16
    x = np.asarray(x, dtype=np.float32)
    y = np.asarray(y, dtype=np.float32)
    xb = x.astype(bf16)                       # [BATCH, DIM]
    yb = y.astype(bf16)
    wxT = np.asarray(Wx, dtype=np.float32).astype(bf16).T  # [DIM, CLASSES]
    wyT = np.asarray(Wy, dtype=np.float32).astype(bf16).T
    KH = KCH // 2

    # [DIM, C] -> [P, KCH, C]  (chunk k occupies rows k*P:(k+1)*P)
    wx3 = np.ascontiguousarray(wxT.reshape(KCH, P, CLASSES).transpose(1, 0, 2))
    wy3 = np.ascontiguousarray(wyT.reshape(KCH, P, CLASSES).transpose(1, 0, 2))

    bxy = np.empty((P, 2, CLASSES), dtype=bf16)
    bxy[:, 0, :] = np.broadcast_to(np.asarray(bx, np.float32).astype(bf16), (P, CLASSES))
    bxy[:, 1, :] = np.broadcast_to(np.asarray(by, np.float32).astype(bf16), (P, CLASSES))

    xT = np.ascontiguousarray(xb.T)           # [DIM, BATCH]
    yT = np.ascontiguousarray(yb.T)
    x4 = xT.reshape(KCH, P, BATCH).transpose(1, 0, 2)   # [P, KCH, BATCH]
    y4 = yT.reshape(KCH, P, BATCH).transpose(1, 0, 2)

    in_maps = []
    for c in range(NCORES):
        rs = slice(c * R, (c + 1) * R)
        xc = x4[:, :, rs]                      # [P, KCH, R]
        yc = y4[:, :, rs]

        px = np.empty((P, KH, 2, CLASSES + TW), dtype=bf16)
        px[:, :, :, :CLASSES] = wx3.reshape(P, KH, 2, CLASSES)
        px[:, :, :, CLASSES:] = xc[:, :, :TW].reshape(P, KH, 2, TW)

        pya = np.empty((P, KH, 2, NH + TW), dtype=bf16)
        pya[:, :, :, :NH] = wy3[:, :, :NH].reshape(P, KH, 2, NH)
        pya[:, :, :, NH:] = yc[:, :, :TW].reshape(P, KH, 2, TW)

        pw1 = np.ascontiguousarray(wy3[:, :, NH:].reshape(P, KCH // 4, 4, NH))
        pxb = np.ascontiguousarray(xc[:, :, TW:].reshape(P, KCH // 4, 4, TW))
        pyb = np.ascontiguousarray(yc[:, :, TW:].reshape(P, KCH // 4, 4, TW))

        in_maps.append(
            {"px": px, "pya": pya, "pw1": pw1, "pxb": pxb, "pyb": pyb, "bxy": bxy}
        )

    res = run_bass_kernel_spmd(nc, in_maps, core_ids=list(range(NCORES)))
    LAST_RESULTS = res

    aa = np.concatenate([res.results[c]["alpha_a"] for c in range(NCORES)], axis=0)
    ax = np.concatenate([res.results[c]["alpha_x"] for c in range(NCORES)], axis=0)
    ay = np.concatenate([res.results[c]["alpha_y"] for c in range(NCORES)], axis=0)
    return (aa, ax, ay)
